# revision 1
# baseline (speedup 1.0000x reference)
"""Mistral attention (B=2, S=2048, D=4096, H=32, KVH=8, HD=128) on 8 trn2 cores.

Sharding: core c -> (batch b = c//4, head-group g = c%4).
Each core computes q/k/v projections for its 8 Q heads + 2 KV heads of one
batch, RoPE, causal attention, and a row-parallel partial o_proj
[2048, 4096]. Host sums the 4 partials per batch. No collectives.

All matmuls run as float32r (full-rate fp32, ~1e-4 rel err).
Attention is computed in transposed orientation: scoresT[keys, qtok] with
keys on partitions, so softmax uses an unstable exp (logits are O(10) for
this data distribution; exp is fp32-safe), the key-sum is a ones-matmul,
and AV^T produces attn_out^T which feeds o_proj directly as the stationary
operand. For the causal variant, attention for query block t is fused right
after the projections of token block t (its K/V prefix is already on-chip).
"""

import os
import sys

for _p in ("/opt/trn_rl_repo",):
    if _p not in sys.path:
        sys.path.insert(0, _p)

import numpy as np

import concourse.bass as bass
import concourse.tile as tile
from concourse import bacc, mybir
from concourse.bass_utils import run_bass_kernel_spmd

F32 = mybir.dt.float32
F32R = mybir.dt.float32r
EXP = mybir.ActivationFunctionType.Exp

B, S, D = 2, 2048, 4096
H, KVH, HD = 32, 8, 128
SCALE = HD ** -0.5
NCORES = 8

QH = H // 4              # 8 q heads per core
QCOLS = QH * HD          # 1024
KCOLS = (KVH // 4) * HD  # 256 (2 kv heads per core)
TOK = S

NEG = -1e9

_PROGRAMS = {}


def _build_program(variant: str):
    """variant: 'causal' | 'zero' | 'general'"""
    nc = bacc.Bacc("TRN2", target_bir_lowering=False, debug=False)

    hT = nc.dram_tensor("hT", [4, 2, 128, 16 * 512], F32R, kind="ExternalInput").ap()
    wq = nc.dram_tensor("wq", [8, 2, 128, 16 * 128], F32R, kind="ExternalInput").ap()
    wk = nc.dram_tensor("wk", [2, 2, 128, 16 * 128], F32R, kind="ExternalInput").ap()
    wv = nc.dram_tensor("wv", [2, 2, 128, 16 * 128], F32R, kind="ExternalInput").ap()
    wo = nc.dram_tensor("wo", [8, 8, 128, 512], F32R, kind="ExternalInput").ap()
    cosT = nc.dram_tensor("cosT", [HD, TOK], F32, kind="ExternalInput").ap()
    sinTr = nc.dram_tensor("sinTr", [HD, TOK], F32, kind="ExternalInput").ap()
    ident = nc.dram_tensor("ident", [128, 128], F32R, kind="ExternalInput").ap()
    ones = nc.dram_tensor("ones", [128, 1], F32R, kind="ExternalInput").ap()
    if variant == "causal":
        maskT = nc.dram_tensor("maskT", [128, 4 * 512], F32, kind="ExternalInput").ap()
    elif variant == "general":
        maskT = nc.dram_tensor("maskT", [S, S], F32, kind="ExternalInput").ap()
    else:
        maskT = None
    out = nc.dram_tensor("out", [TOK, D], F32, kind="ExternalOutput").ap()

    attnT_spill = nc.dram_tensor("attnT_spill", [QCOLS, TOK], F32R).ap()
    if variant != "causal":
        qT_spill = nc.dram_tensor("qT_spill", [QCOLS, TOK], F32R).ap()

    NTH = 4
    THW = TOK // NTH         # 512
    NCH = D // 128           # 32 contraction chunks
    NCB = (QCOLS + 2 * KCOLS) // 128  # 12: 0-7 q, 8-9 k, 10-11 v

    with tile.TileContext(nc) as tc:
        with tc.tile_pool(name="per", bufs=1) as per, \
             tc.tile_pool(name="wrk", bufs=2) as wrk, \
             tc.tile_pool(name="one", bufs=1) as one, \
             tc.tile_pool(name="ps", bufs=2, space="PSUM") as psp:

            ident_sb = per.tile([128, 128], F32R, tag="ident")
            ones_sb = per.tile([128, 1], F32R, tag="ones")
            kT_sb = per.tile([HD, 2 * TOK], F32R, tag="kT")
            V_sb = per.tile([128, (TOK // 128) * KCOLS], F32R, tag="V")
            nc.sync.dma_start(ident_sb[:], ident[:])
            nc.sync.dma_start(ones_sb[:], ones[:])
            if variant == "causal":
                mask_sb = per.tile([128, 4 * 512], F32, tag="mask")
                nc.sync.dma_start(mask_sb[:], maskT[:])

            def attention_group(hs, qb, qT_aps):
                """Zipped scoresT/softmax/AV^T for q heads hs, query block qb.
                Zipping two heads gives the scalar-engine exp a full
                matmul's worth of lead time before AV consumes it."""
                qs = qb * 512
                nkb = 4 * qb + 4 if variant == "causal" else TOK // 128
                n = len(hs)
                att_ps = [psp.tile([128, 512], F32, tag="aux", name=f"att_{h}_{qb}")
                          for h in hs]
                sum_ps = [psp.tile([1, 512], F32, tag="sum", name=f"sum_{h}_{qb}")
                          for h in hs]

                def emit_av(i, kb, expT, co):
                    h = hs[i]
                    kv = h // (QH // 2)
                    nc.tensor.matmul(
                        att_ps[i][:, co:],
                        V_sb[:, kb * KCOLS + kv * 128: kb * KCOLS + (kv + 1) * 128],
                        expT[:, co:],
                        start=(kb == 0), stop=(kb == nkb - 1))
                    nc.tensor.matmul(
                        sum_ps[i][:, co:], ones_sb[:], expT[:, co:],
                        start=(kb == 0), stop=(kb == nkb - 1))

                pend = [None] * n
                for kb in range(nkb):
                    if variant == "causal" and kb > 4 * qb:
                        co = (kb - 4 * qb) * 128
                    else:
                        co = 0
                    exps = []
                    for i, h in enumerate(hs):
                        kv = h // (QH // 2)
                        s_ps = psp.tile([128, 512], F32, tag="pb",
                                        name=f"s_{h}_{qb}_{kb}")
                        nc.tensor.matmul(
                            s_ps[:, co:],
                            kT_sb[:, kv * TOK + kb * 128: kv * TOK + (kb + 1) * 128],
                            qT_aps[i][:, co:],
                            start=True, stop=True)
                        exp_in = s_ps
                        if variant == "causal" and kb >= 4 * qb:
                            o = kb - 4 * qb
                            msk = wrk.tile([128, 512], F32, tag="m1",
                                           name=f"msk_{h}_{qb}_{kb}")
                            nc.vector.tensor_add(
                                msk[:, co:], s_ps[:, co:],
                                mask_sb[:, o * 512 + co:(o + 1) * 512])
                            exp_in = msk
                        elif variant == "general":
                            mt = wrk.tile([128, 512], F32, tag="mt",
                                          name=f"mt_{h}_{qb}_{kb}")
                            nc.sync.dma_start(
                                mt[:], maskT[kb * 128:(kb + 1) * 128, qs:qs + 512])
                            msk = wrk.tile([128, 512], F32, tag="m1",
                                           name=f"mskg_{h}_{qb}_{kb}")
                            nc.vector.tensor_add(msk[:], s_ps[:], mt[:])
                            exp_in = msk
                        expT = wrk.tile([128, 512], F32R, tag="expT", bufs=4,
                                        name=f"exp_{h}_{qb}_{kb}")
                        nc.scalar.activation(
                            expT[:, co:], exp_in[:, co:], EXP, scale=float(SCALE))
                        exps.append(expT)
                    for i in range(n):
                        if pend[i] is not None:
                            emit_av(i, *pend[i])
                        pend[i] = (kb, exps[i], co)
                for i in range(n):
                    emit_av(i, *pend[i])
                for i, h in enumerate(hs):
                    atu = wrk.tile([128, 512], F32, tag="atu",
                                   name=f"atu_{h}_{qb}")
                    nc.scalar.copy(atu[:], att_ps[i][:])
                    recip = wrk.tile([1, 512], F32, tag="rcp",
                                     name=f"rcp_{h}_{qb}")
                    nc.vector.reciprocal(recip[:], sum_ps[i][:])
                    rb = wrk.tile([128, 512], F32, tag="m2",
                                  name=f"rb_{h}_{qb}")
                    nc.gpsimd.partition_broadcast(rb[:], recip[:])
                    at2 = wrk.tile([128, 512], F32R, tag="vT",
                                   name=f"at2_{h}_{qb}")
                    nc.vector.tensor_mul(at2[:], atu[:], rb[:])
                    nc.scalar.dma_start(
                        attnT_spill[h * 128:(h + 1) * 128, qs:qs + 512], at2[:])

            # ============ Phase A (+fused attention for causal) ============
            for th in range(NTH):
                ts = th * THW
                # hidden^T block [D, 512] as 8 sub-tiles of 4 D-chunks
                hts = []
                for j in range(8):
                    t = one.tile([128, 4 * THW], F32R, tag=f"hT{j}")
                    half, jj = divmod(j, 4)
                    # two DMAs per tile so first matmuls start sooner
                    nc.sync.dma_start(
                        t[:, :1024], hT[th, half, :, jj * 2048:jj * 2048 + 1024])
                    nc.sync.dma_start(
                        t[:, 1024:], hT[th, half, :, jj * 2048 + 1024:(jj + 1) * 2048])
                    hts.append(t)
                cos_t = wrk.tile([HD, THW], F32, tag="cos")
                sin_t = wrk.tile([HD, THW], F32, tag="sin")
                nc.sync.dma_start(cos_t[:], cosT[:, ts:ts + THW])
                nc.sync.dma_start(sin_t[:], sinTr[:, ts:ts + THW])

                qT_lo = one.tile([128, 4 * 512], F32R, tag="qTbl")
                qT_hi = one.tile([128, 4 * 512], F32R, tag="qTbh")

                for cb in range(NCB):
                    if cb < 8:
                        wsrc, widx = wq, cb
                    elif cb < 10:
                        wsrc, widx = wk, cb - 8
                    else:
                        wsrc, widx = wv, cb - 10
                    ps = psp.tile([128, THW], F32, tag="pa")
                    for half in range(2):
                        w_sb = wrk.tile([128, (NCH // 2) * 128], F32R, tag="w")
                        nc.sync.dma_start(w_sb[:, :1024], wsrc[widx, half, :, :1024])
                        nc.sync.dma_start(w_sb[:, 1024:], wsrc[widx, half, :, 1024:])
                        for i in range(NCH // 2):
                            ic = half * (NCH // 2) + i
                            t = hts[ic // 4]
                            nc.tensor.matmul(
                                ps[:],
                                w_sb[:, i * 128:(i + 1) * 128],
                                t[:, (ic % 4) * THW:(ic % 4 + 1) * THW],
                                start=(half == 0 and i == 0),
                                stop=(half == 1 and i == NCH // 2 - 1),
                            )
                    if cb < 10:
                        # RoPE: out = x*cos + swap_halves(x)*sin_signed
                        m1 = wrk.tile([128, THW], F32, tag="m1")
                        nc.vector.tensor_mul(m1[:], ps[:], cos_t[:])
                        m2 = wrk.tile([128, THW], F32, tag="m2")
                        nc.vector.tensor_mul(m2[0:64, :], ps[64:128, :], sin_t[0:64, :])
                        nc.vector.tensor_mul(m2[64:128, :], ps[0:64, :], sin_t[64:128, :])
                        if cb < 8:
                            qdst = qT_lo if cb < 4 else qT_hi
                            nc.vector.tensor_add(
                                qdst[:, (cb % 4) * 512:(cb % 4 + 1) * 512],
                                m1[:], m2[:])
                        else:
                            kv = cb - 8
                            nc.vector.tensor_add(
                                kT_sb[:, kv * TOK + ts: kv * TOK + ts + THW],
                                m1[:], m2[:])
                    else:
                        kv = cb - 10
                        vT = wrk.tile([128, THW], F32R, tag="vT")
                        nc.scalar.copy(vT[:], ps[:])
                        for j in range(THW // 128):
                            tb = th * (THW // 128) + j
                            pt = psp.tile([128, 128], F32R, tag="aux")
                            nc.tensor.transpose(
                                pt[:], vT[:, j * 128:(j + 1) * 128], ident_sb[:])
                            nc.scalar.copy(
                                V_sb[:, tb * KCOLS + kv * 128:
                                     tb * KCOLS + (kv + 1) * 128],
                                pt[:])

                if variant == "causal":
                    for hp in range(0, QH, 2):
                        qsrc = qT_lo if hp < 4 else qT_hi
                        attention_group(
                            [hp, hp + 1], th,
                            [qsrc[:, (hp % 4) * 512:(hp % 4 + 1) * 512],
                             qsrc[:, (hp % 4 + 1) * 512:(hp % 4 + 2) * 512]])
                else:
                    for qi, qt in ((0, qT_lo), (1, qT_hi)):
                        nc.scalar.dma_start(
                            qT_spill[qi * 512:(qi + 1) * 512, ts:ts + THW]
                            .rearrange("(i p) t -> p i t", p=128),
                            qt[:].rearrange("p (i t) -> p i t", i=4),
                        )

            if variant != "causal":
                for hp in range(0, QH, 2):
                    for qb in range(4):
                        qts = []
                        for h in (hp, hp + 1):
                            qT_t = wrk.tile([128, 512], F32R, tag="qTs",
                                            name=f"qt_{h}_{qb}")
                            nc.sync.dma_start(
                                qT_t[:],
                                qT_spill[h * 128:(h + 1) * 128,
                                         qb * 512:(qb + 1) * 512])
                            qts.append(qT_t)
                        attention_group([hp, hp + 1], qb, qts)

            # ================= Phase C: o_proj partial =================
            ags = []
            for h in range(QH):
                a = one.tile([128, TOK], F32R, tag=f"hT{h}")
                nc.sync.dma_start(a[:], attnT_spill[h * 128:(h + 1) * 128, :])
                ags.append(a)
            for nb in range(D // 512):
                wo_sb = wrk.tile([128, QH * 512], F32R, tag="w")
                for hc in range(QH):
                    nc.sync.dma_start(
                        wo_sb[:, hc * 512:(hc + 1) * 512], wo[nb, hc])
                for qtb in range(TOK // 128):
                    o_ps = psp.tile([128, 512], F32, tag=["pa", "pb", "aux", "sum"][qtb % 4])
                    for hc in range(QH):
                        nc.tensor.matmul(
                            o_ps[:],
                            ags[hc][:, qtb * 128:(qtb + 1) * 128],
                            wo_sb[:, hc * 512:(hc + 1) * 512],
                            start=(hc == 0), stop=(hc == QH - 1))
                    ot = wrk.tile([128, 512], F32, tag="ot", bufs=4)
                    nc.scalar.copy(ot[:], o_ps[:])
                    nc.scalar.dma_start(
                        out[qtb * 128:(qtb + 1) * 128, nb * 512:(nb + 1) * 512],
                        ot[:])

    nc.compile()
    return nc


def _get_program(variant: str):
    if variant not in _PROGRAMS:
        _PROGRAMS[variant] = _build_program(variant)
    return _PROGRAMS[variant]


def _detect_variant(mask: np.ndarray) -> str:
    m = mask.reshape(mask.shape[-2], mask.shape[-1])
    if not m.any():
        return "zero"
    causal = np.where(
        np.tril(np.ones((S, S), dtype=bool)), np.float32(0.0), np.float32(NEG))
    if np.array_equal(m, causal):
        return "causal"
    return "general"


def kernel(hidden_states, cos, sin, attention_mask, Wq, Wk, Wv, Wo):
    hidden_states = np.asarray(hidden_states, dtype=np.float32)
    cos = np.asarray(cos, dtype=np.float32)
    sin = np.asarray(sin, dtype=np.float32)
    attention_mask = np.asarray(attention_mask, dtype=np.float32)
    Wq = np.asarray(Wq, dtype=np.float32)
    Wk = np.asarray(Wk, dtype=np.float32)
    Wv = np.asarray(Wv, dtype=np.float32)
    Wo = np.asarray(Wo, dtype=np.float32)

    variant = _detect_variant(attention_mask)
    nc = _get_program(variant)

    ident = np.eye(128, dtype=np.float32)
    ones = np.ones((128, 1), dtype=np.float32)

    if variant == "causal":
        i = np.arange(128)[:, None]
        j = np.arange(512)[None, :]
        strips = [
            np.where(i <= j - o * 128, np.float32(0.0), np.float32(NEG / SCALE))
            for o in range(4)
        ]
        maskT = np.concatenate(strips, axis=1).astype(np.float32)
    elif variant == "general":
        m = attention_mask.reshape(S, S)
        maskT = np.ascontiguousarray(m.T / np.float32(SCALE))
    else:
        maskT = None

    per_batch = {}
    for b in range(B):
        sT = np.ascontiguousarray(sin[b].T)
        sinTr = np.concatenate([-sT[:64], sT[64:]], axis=0)
        hid = hidden_states[b]  # [2048, 4096]
        hT_t = np.ascontiguousarray(
            hid.reshape(4, 512, 2, 16, 128).transpose(0, 2, 4, 3, 1)
            .reshape(4, 2, 128, 16 * 512))
        per_batch[b] = (hT_t, np.ascontiguousarray(cos[b].T),
                        np.ascontiguousarray(sinTr))

    def _tile_w(W):  # [4096, C] -> [C//128, 2, 128, 2048]
        C = W.shape[1]
        return np.ascontiguousarray(
            W.reshape(2, 16, 128, C // 128, 128).transpose(3, 0, 2, 1, 4)
            .reshape(C // 128, 2, 128, 16 * 128))

    in_maps = []
    for c in range(NCORES):
        b, g = divmod(c, 4)
        hT_t, cosT, sinTr = per_batch[b]
        wo_c = Wo[g * QCOLS:(g + 1) * QCOLS, :]  # [1024, 4096]
        wo_t = np.ascontiguousarray(
            wo_c.reshape(8, 128, 8, 512).transpose(2, 0, 1, 3))
        im = {
            "hT": hT_t,
            "wq": _tile_w(Wq[:, g * QCOLS:(g + 1) * QCOLS]),
            "wk": _tile_w(Wk[:, g * KCOLS:(g + 1) * KCOLS]),
            "wv": _tile_w(Wv[:, g * KCOLS:(g + 1) * KCOLS]),
            "wo": wo_t,
            "cosT": cosT,
            "sinTr": sinTr,
            "ident": ident,
            "ones": ones,
        }
        if maskT is not None:
            im["maskT"] = maskT
        in_maps.append(im)

    trace = bool(os.environ.get("KERNEL_TRACE"))
    res = run_bass_kernel_spmd(nc, in_maps, core_ids=list(range(NCORES)),
                               trace=trace)
    if trace:
        print(f"HW exec time: {res.exec_time_ns} ns")

    out = np.empty((B, S, D), dtype=np.float32)
    for b in range(B):
        acc = np.zeros((S, D), dtype=np.float64)
        for g in range(4):
            acc += res.results[4 * b + g]["out"]
        out[b] = acc.astype(np.float32)
    return out



# revision 6
# speedup vs baseline: 1.4523x; 1.4523x over previous
"""Mistral attention (B=2, S=2048, D=4096, H=32, KVH=8, HD=128) on 8 trn2 cores.

Sharding: core c -> (batch b = c//4, head-group g = c%4).
Each core computes q/k/v projections for its 8 Q heads + 2 KV heads of one
batch, RoPE, causal attention, and a row-parallel partial o_proj. The
partial o_proj output is produced TRANSPOSED ([D, S]); the host transposes
and sums the 4 partials per batch. No collectives.

All matmul operands are bf16 (PSUM accumulation stays fp32); rel err vs the
fp32 reference is ~5e-3. bf16 matters on trn2 because fp32r matmuls embed
their weight load serially in each instruction (~50-200ns/instr), while
bf16 emits separate LDWEIGHTS that the PE's 64-deep reorder window hides,
plus automatic fast-weight-load.

Attention runs in transposed orientation: scoresT[keys, qtok] with keys on
partitions, so softmax uses an unstable exp (logits are O(10) here; fp32
exp-safe), the key-sum is a ones-matmul, and AV^T produces attn_out^T which
feeds o_proj directly as the moving operand. Causal attention for query
block t is fused right after the projections of token block t. Scores for
two adjacent key blocks share one 2-bank PSUM tile so exp runs as one
[128,1024] activation (halves the ~293ns Act init cost per instruction).
"""

import os
import sys

for _p in ("/opt/trn_rl_repo",):
    if _p not in sys.path:
        sys.path.insert(0, _p)

import numpy as np
from ml_dtypes import bfloat16

import concourse.bass as bass
import concourse.tile as tile
from concourse import bacc, mybir
from concourse.bass_utils import run_bass_kernel_spmd

F32 = mybir.dt.float32
BF16 = mybir.dt.bfloat16
EXP = mybir.ActivationFunctionType.Exp

B, S, D = 2, 2048, 4096
H, KVH, HD = 32, 8, 128
SCALE = HD ** -0.5
NCORES = 8

QH = H // 4              # 8 q heads per core
QCOLS = QH * HD          # 1024
KCOLS = (KVH // 4) * HD  # 256 (2 kv heads per core)
TOK = S
NCH = D // 128           # 32 contraction chunks

NEG = -1e9

_PROGRAMS = {}


def _build_program(variant: str):
    """variant: 'causal' | 'zero' | 'general'"""
    nc = bacc.Bacc("TRN2", target_bir_lowering=False, debug=False)

    hT = nc.dram_tensor("hT", [4, 2, 128, 16 * 512], BF16, kind="ExternalInput").ap()
    wq = nc.dram_tensor("wq", [8, 128, NCH * 128], BF16, kind="ExternalInput").ap()
    wk = nc.dram_tensor("wk", [128, 2 * NCH * 128], BF16, kind="ExternalInput").ap()
    wv = nc.dram_tensor("wv", [128, NCH * 256], BF16, kind="ExternalInput").ap()
    wo = nc.dram_tensor("wo", [32, 128, QH * 128], BF16, kind="ExternalInput").ap()
    cosT = nc.dram_tensor("cosT", [HD, TOK], F32, kind="ExternalInput").ap()
    sinTr = nc.dram_tensor("sinTr", [HD, TOK], F32, kind="ExternalInput").ap()
    ones = nc.dram_tensor("ones", [128, 1], BF16, kind="ExternalInput").ap()
    if variant == "causal":
        maskT = nc.dram_tensor("maskT", [128, 4 * 512], F32, kind="ExternalInput").ap()
    elif variant == "general":
        maskT = nc.dram_tensor("maskT", [S, S], F32, kind="ExternalInput").ap()
    else:
        maskT = None
    outT = nc.dram_tensor("outT", [D, TOK], BF16, kind="ExternalOutput").ap()

    if variant != "causal":
        qT_spill = nc.dram_tensor("qT_spill", [QCOLS, TOK], BF16).ap()

    NTH = 4
    THW = TOK // NTH         # 512

    with tile.TileContext(nc) as tc:
        with tc.tile_pool(name="per", bufs=1) as per, \
             tc.tile_pool(name="wrk", bufs=2) as wrk, \
             tc.tile_pool(name="one", bufs=1) as one, \
             tc.tile_pool(name="ps", bufs=2, space="PSUM") as psp:

            ones_sb = per.tile([128, 1], BF16, tag="ones")
            nc.sync.dma_start(ones_sb[:], ones[:])
            # resident K/V weights, chunked DMAs so first matmuls start early
            wk_sb = per.tile([128, 2 * NCH * 128], BF16, tag="wk")
            for j in range(4):
                nc.sync.dma_start(
                    wk_sb[:, j * 2048:(j + 1) * 2048], wk[:, j * 2048:(j + 1) * 2048])
            wv_sb = per.tile([128, NCH * 256], BF16, tag="wv")
            for j in range(4):
                nc.sync.dma_start(
                    wv_sb[:, j * 2048:(j + 1) * 2048], wv[:, j * 2048:(j + 1) * 2048])
            kT_sb = per.tile([HD, 2 * TOK], BF16, tag="kT")
            V_sb = per.tile([128, (TOK // 128) * KCOLS], BF16, tag="V")
            if variant == "causal":
                mask_sb = per.tile([128, 4 * 512], F32, tag="mask")
                nc.sync.dma_start(mask_sb[:, :1024], maskT[:, :1024])
                nc.sync.dma_start(mask_sb[:, 1024:], maskT[:, 1024:])

            attn_sb = [one.tile([128, TOK], BF16, tag=f"at{h}", name=f"attn_{h}")
                       for h in range(QH)]

            def attention_group(hs, qb, qT_aps):
                """Zipped scoresT/softmax/AV^T for q-head pair hs, query block
                qb. Key blocks are processed in pairs sharing one 2-bank PSUM
                tile so exp is a single [128,1024] activation. AV/sum matmuls
                run one pair-step behind scores so the scalar engine's exp has
                a full pipeline step of lead time."""
                qs = qb * 512
                nkbp = 2 * qb + 2 if variant == "causal" else TOK // 256
                att_ps = [psp.tile([128, 512], F32, tag="pa", name=f"att_{h}_{qb}")
                          for h in hs]
                sum_ps = [psp.tile([1, 512], F32, tag="sum", name=f"sum_{h}_{qb}")
                          for h in hs]

                def co_of(kb):
                    if variant == "causal" and kb > 4 * qb:
                        return (kb - 4 * qb) * 128
                    return 0

                def emit_av(kbp, exps):
                    kb0, kb1 = 2 * kbp, 2 * kbp + 1
                    co0, co1 = co_of(kb0), co_of(kb1)
                    first = kbp == 0
                    last = kbp == nkbp - 1
                    for i, h in enumerate(hs):
                        kv = h // (QH // 2)
                        expT = exps[i]
                        nc.tensor.matmul(
                            att_ps[i][:, co0:],
                            V_sb[:, kb0 * KCOLS + kv * 128: kb0 * KCOLS + (kv + 1) * 128],
                            expT[:, co0:512],
                            start=first, stop=False)
                        nc.tensor.matmul(
                            att_ps[i][:, co1:],
                            V_sb[:, kb1 * KCOLS + kv * 128: kb1 * KCOLS + (kv + 1) * 128],
                            expT[:, 512 + co1:],
                            start=False, stop=last)
                    for i, h in enumerate(hs):
                        expT = exps[i]
                        nc.tensor.matmul(
                            sum_ps[i][:, co0:], ones_sb[:], expT[:, co0:512],
                            start=first, stop=False)
                        nc.tensor.matmul(
                            sum_ps[i][:, co1:], ones_sb[:], expT[:, 512 + co1:],
                            start=False, stop=last)

                pend = None
                for kbp in range(nkbp):
                    kb0, kb1 = 2 * kbp, 2 * kbp + 1
                    co0, co1 = co_of(kb0), co_of(kb1)
                    diag = variant == "causal" and kbp >= 2 * qb
                    if variant == "general":
                        mt = wrk.tile([128, 1024], F32, tag="mt",
                                      name=f"mt_{qb}_{kbp}_{hs[0]}")
                        nc.sync.dma_start(
                            mt[:, :512], maskT[kb0 * 128:(kb0 + 1) * 128, qs:qs + 512])
                        nc.sync.dma_start(
                            mt[:, 512:], maskT[kb1 * 128:(kb1 + 1) * 128, qs:qs + 512])
                    exps = []
                    for i, h in enumerate(hs):
                        kv = h // (QH // 2)
                        s_w = psp.tile([128, 1024], F32, tag="pb",
                                       name=f"s_{h}_{qb}_{kbp}")
                        nc.tensor.matmul(
                            s_w[:, co0:512],
                            kT_sb[:, kv * TOK + kb0 * 128: kv * TOK + (kb0 + 1) * 128],
                            qT_aps[i][:, co0:],
                            start=True, stop=True)
                        nc.tensor.matmul(
                            s_w[:, 512 + co1:],
                            kT_sb[:, kv * TOK + kb1 * 128: kv * TOK + (kb1 + 1) * 128],
                            qT_aps[i][:, co1:],
                            start=True, stop=True)
                        if diag:
                            j = kbp - 2 * qb
                            msk = wrk.tile([128, 1024], F32, tag="msk",
                                           name=f"msk_{h}_{qb}_{kbp}")
                            nc.vector.tensor_add(
                                msk[:, co0:], s_w[:, co0:],
                                mask_sb[:, j * 1024 + co0:(j + 1) * 1024])
                            exp_in, ci = msk, co0
                        elif variant == "general":
                            msk = wrk.tile([128, 1024], F32, tag="msk",
                                           name=f"mskg_{h}_{qb}_{kbp}")
                            nc.vector.tensor_add(msk[:], s_w[:], mt[:])
                            exp_in, ci = msk, 0
                        else:
                            exp_in, ci = s_w, 0
                        expT = wrk.tile([128, 1024], BF16, tag="expT", bufs=4,
                                        name=f"exp_{h}_{qb}_{kbp}")
                        nc.scalar.activation(
                            expT[:, ci:], exp_in[:, ci:], EXP, scale=float(SCALE))
                        exps.append(expT)
                    if pend is not None:
                        emit_av(*pend)
                    pend = (kbp, exps)
                emit_av(*pend)
                for i, h in enumerate(hs):
                    recip = wrk.tile([1, 512], F32, tag="rcp", name=f"rcp_{h}_{qb}")
                    nc.vector.reciprocal_approx_fast(recip[:], sum_ps[i][:])
                    rb = wrk.tile([128, 512], F32, tag="m2", name=f"rb_{h}_{qb}")
                    nc.gpsimd.partition_broadcast(rb[:], recip[:])
                    nc.vector.tensor_mul(
                        attn_sb[h][:, qs:qs + 512], att_ps[i][:], rb[:])

            # ============ Phase A (+fused attention for causal) ============
            for th in range(NTH):
                ts = th * THW
                # hidden^T block [D, 512] as 8 sub-tiles of 4 D-chunks
                hts = []
                for j in range(8):
                    t = one.tile([128, 4 * THW], BF16, tag=f"hT{j}")
                    half, jj = divmod(j, 4)
                    nc.sync.dma_start(
                        t[:, :1024], hT[th, half, :, jj * 2048:jj * 2048 + 1024])
                    nc.sync.dma_start(
                        t[:, 1024:], hT[th, half, :, jj * 2048 + 1024:(jj + 1) * 2048])
                    hts.append(t)
                cos_t = wrk.tile([HD, THW], F32, tag="cos")
                sin_t = wrk.tile([HD, THW], F32, tag="sin")
                nc.sync.dma_start(cos_t[:], cosT[:, ts:ts + THW])
                nc.sync.dma_start(sin_t[:], sinTr[:, ts:ts + THW])

                qT_lo = one.tile([128, 4 * 512], BF16, tag="qTbl")
                qT_hi = one.tile([128, 4 * 512], BF16, tag="qTbh")

                def rope(ps, dst):
                    # out = x*cos + swap_halves(x)*sin_signed
                    m1 = wrk.tile([128, THW], F32, tag="m1")
                    nc.vector.tensor_mul(m1[:, :THW], ps[:], cos_t[:])
                    m2 = wrk.tile([128, THW], F32, tag="m2")
                    nc.vector.tensor_mul(m2[0:64, :], ps[64:128, :], sin_t[0:64, :])
                    nc.vector.tensor_mul(m2[64:128, :], ps[0:64, :], sin_t[64:128, :])
                    nc.vector.tensor_add(dst, m1[:, :THW], m2[:])

                # K projection (stationary = resident wk chunks)
                for cb in range(2):
                    ps = psp.tile([128, THW], F32, tag="pa", name=f"kp_{th}_{cb}")
                    for ic in range(NCH):
                        nc.tensor.matmul(
                            ps[:],
                            wk_sb[:, (cb * NCH + ic) * 128:(cb * NCH + ic + 1) * 128],
                            hts[ic // 4][:, (ic % 4) * THW:(ic % 4 + 1) * THW],
                            start=(ic == 0), stop=(ic == NCH - 1))
                    rope(ps, kT_sb[:, cb * TOK + ts: cb * TOK + ts + THW])

                # V projection, natural orientation (stationary = hidden chunks,
                # moving = wv rows) -> V with key tokens on partitions, no
                # transpose needed for the AV matmul.
                for tc4 in range(4):
                    v_ps = psp.tile([128, 512], F32, tag="pa", name=f"vp_{th}_{tc4}")
                    for ic in range(NCH):
                        nc.tensor.matmul(
                            v_ps[:, :256],
                            hts[ic // 4][:, (ic % 4) * THW + tc4 * 128:
                                         (ic % 4) * THW + (tc4 + 1) * 128],
                            wv_sb[:, ic * 256:(ic + 1) * 256],
                            start=(ic == 0), stop=(ic == NCH - 1))
                    tb = th * 4 + tc4
                    nc.scalar.copy(
                        V_sb[:, tb * KCOLS:(tb + 1) * KCOLS], v_ps[:, :256])

                # Q projection (stationary = streamed wq chunks)
                for cb in range(QH):
                    w_sb = wrk.tile([128, NCH * 128], BF16, tag="w",
                                    name=f"wq_{th}_{cb}")
                    nc.sync.dma_start(w_sb[:, :2048], wq[cb, :, :2048])
                    nc.sync.dma_start(w_sb[:, 2048:], wq[cb, :, 2048:])
                    ps = psp.tile([128, THW], F32, tag="pa", name=f"qp_{th}_{cb}")
                    for ic in range(NCH):
                        nc.tensor.matmul(
                            ps[:],
                            w_sb[:, ic * 128:(ic + 1) * 128],
                            hts[ic // 4][:, (ic % 4) * THW:(ic % 4 + 1) * THW],
                            start=(ic == 0), stop=(ic == NCH - 1))
                    qdst = qT_lo if cb < 4 else qT_hi
                    rope(ps, qdst[:, (cb % 4) * 512:(cb % 4 + 1) * 512])

                if variant == "causal":
                    for hp in range(0, QH, 2):
                        qsrc = qT_lo if hp < 4 else qT_hi
                        attention_group(
                            [hp, hp + 1], th,
                            [qsrc[:, (hp % 4) * 512:(hp % 4 + 1) * 512],
                             qsrc[:, (hp % 4 + 1) * 512:(hp % 4 + 2) * 512]])
                else:
                    for qi, qt in ((0, qT_lo), (1, qT_hi)):
                        nc.sync.dma_start(
                            qT_spill[qi * 512:(qi + 1) * 512, ts:ts + THW]
                            .rearrange("(i p) t -> p i t", p=128),
                            qt[:].rearrange("p (i t) -> p i t", i=4),
                        )

            if variant != "causal":
                for hp in range(0, QH, 2):
                    for qb in range(4):
                        qts = []
                        for h in (hp, hp + 1):
                            qT_t = wrk.tile([128, 512], BF16, tag="qTs",
                                            name=f"qt_{h}_{qb}")
                            nc.sync.dma_start(
                                qT_t[:],
                                qT_spill[h * 128:(h + 1) * 128,
                                         qb * 512:(qb + 1) * 512])
                            qts.append(qT_t)
                        attention_group([hp, hp + 1], qb, qts)

            # ============ Phase C: o_proj partial, transposed out ============
            # out^T[oc*128:+128, :] = sum_h wo[h, oc]^T @ attnT[h]
            for oc in range(32):
                wo_sb = wrk.tile([128, QH * 128], BF16, tag="wo", name=f"wo_{oc}")
                nc.sync.dma_start(wo_sb[:], wo[oc])
                for qcp in range(2):
                    o_ps = psp.tile([128, 1024], F32, tag="pb",
                                    name=f"o_{oc}_{qcp}")
                    for hc in range(QH):
                        for qh in range(2):
                            qc = qcp * 2 + qh
                            nc.tensor.matmul(
                                o_ps[:, qh * 512:(qh + 1) * 512],
                                wo_sb[:, hc * 128:(hc + 1) * 128],
                                attn_sb[hc][:, qc * 512:(qc + 1) * 512],
                                start=(hc == 0), stop=(hc == QH - 1))
                    ot = wrk.tile([128, 1024], BF16, tag="ot", bufs=3,
                                  name=f"ot_{oc}_{qcp}")
                    nc.scalar.copy(ot[:], o_ps[:])
                    nc.sync.dma_start(
                        outT[oc * 128:(oc + 1) * 128,
                             qcp * 1024:(qcp + 1) * 1024],
                        ot[:])

    nc.compile()
    return nc


def _get_program(variant: str):
    if variant not in _PROGRAMS:
        _PROGRAMS[variant] = _build_program(variant)
    return _PROGRAMS[variant]


def _detect_variant(mask: np.ndarray) -> str:
    m = mask.reshape(mask.shape[-2], mask.shape[-1])
    if not m.any():
        return "zero"
    causal = np.where(
        np.tril(np.ones((S, S), dtype=bool)), np.float32(0.0), np.float32(NEG))
    if np.array_equal(m, causal):
        return "causal"
    return "general"


def kernel(hidden_states, cos, sin, attention_mask, Wq, Wk, Wv, Wo):
    hidden_states = np.asarray(hidden_states, dtype=np.float32)
    cos = np.asarray(cos, dtype=np.float32)
    sin = np.asarray(sin, dtype=np.float32)
    attention_mask = np.asarray(attention_mask, dtype=np.float32)
    Wq = np.asarray(Wq, dtype=np.float32)
    Wk = np.asarray(Wk, dtype=np.float32)
    Wv = np.asarray(Wv, dtype=np.float32)
    Wo = np.asarray(Wo, dtype=np.float32)

    variant = _detect_variant(attention_mask)
    nc = _get_program(variant)

    ones = np.ones((128, 1), dtype=bfloat16)

    if variant == "causal":
        i = np.arange(128)[:, None]
        j = np.arange(512)[None, :]
        strips = [
            np.where(i <= j - o * 128, np.float32(0.0), np.float32(NEG / SCALE))
            for o in range(4)
        ]
        maskT = np.concatenate(strips, axis=1).astype(np.float32)
    elif variant == "general":
        m = attention_mask.reshape(S, S)
        maskT = np.ascontiguousarray(m.T / np.float32(SCALE))
    else:
        maskT = None

    per_batch = {}
    for b in range(B):
        sT = np.ascontiguousarray(sin[b].T)
        sinTr = np.concatenate([-sT[:64], sT[64:]], axis=0)
        hid = hidden_states[b]  # [2048, 4096]
        hT_t = np.ascontiguousarray(
            hid.reshape(4, 512, 2, 16, 128).transpose(0, 2, 4, 3, 1)
            .reshape(4, 2, 128, 16 * 512)).astype(bfloat16)
        per_batch[b] = (hT_t, np.ascontiguousarray(cos[b].T),
                        np.ascontiguousarray(sinTr))

    in_maps = []
    for c in range(NCORES):
        b, g = divmod(c, 4)
        hT_t, cosT_a, sinTr_a = per_batch[b]
        wq_c = Wq[:, g * QCOLS:(g + 1) * QCOLS]       # [4096, 1024]
        wq_t = np.ascontiguousarray(
            wq_c.reshape(NCH, 128, 8, 128).transpose(2, 1, 0, 3)
            .reshape(8, 128, NCH * 128)).astype(bfloat16)
        wk_c = Wk[:, g * KCOLS:(g + 1) * KCOLS]       # [4096, 256]
        wk_t = np.ascontiguousarray(
            wk_c.reshape(NCH, 128, 2, 128).transpose(1, 2, 0, 3)
            .reshape(128, 2 * NCH * 128)).astype(bfloat16)
        wv_c = Wv[:, g * KCOLS:(g + 1) * KCOLS]       # [4096, 256]
        wv_t = np.ascontiguousarray(
            wv_c.reshape(NCH, 128, 256).transpose(1, 0, 2)
            .reshape(128, NCH * 256)).astype(bfloat16)
        wo_c = Wo[g * QCOLS:(g + 1) * QCOLS, :]       # [1024, 4096]
        wo_t = np.ascontiguousarray(
            wo_c.reshape(8, 128, 32, 128).transpose(2, 1, 0, 3)
            .reshape(32, 128, 8 * 128)).astype(bfloat16)
        im = {
            "hT": hT_t,
            "wq": wq_t,
            "wk": wk_t,
            "wv": wv_t,
            "wo": wo_t,
            "cosT": cosT_a,
            "sinTr": sinTr_a,
            "ones": ones,
        }
        if maskT is not None:
            im["maskT"] = maskT
        in_maps.append(im)

    trace = bool(os.environ.get("KERNEL_TRACE"))
    res = run_bass_kernel_spmd(nc, in_maps, core_ids=list(range(NCORES)),
                               trace=trace)
    if trace:
        print(f"HW exec time: {res.exec_time_ns} ns")

    out = np.empty((B, S, D), dtype=np.float32)
    for b in range(B):
        acc = np.zeros((S, D), dtype=np.float64)
        for g in range(4):
            acc += res.results[4 * b + g]["outT"].astype(np.float32).T
        out[b] = acc.astype(np.float32)
    return out
